# revision 16
# baseline (speedup 1.0000x reference)
"""Trainium2 Bass kernel for nn_BaseMLP (per-node GNN message-passing MLP).

Reference computation (D=256 nodes, HID=64, P=2, BS=1024):
    xmask[b,j,t] = M[b,j,t] * adj[j,t] * x[b,j]
    h   = lrelu(einsum('tij,bjt->bti', W0, xmask) + b0)
    h   = lrelu(einsum('tij,btj->bti', W1, h) + b1)
    out =        einsum('tij,btj->bti', W2, h) + b2

Sharding: model-parallel over the target-node dim t — each of the 8 cores
owns 32 t's. Per-core M traffic matches batch sharding (M/8) while
replicated-weight traffic drops 8x.

Host-side prep (layout + dtype only, plus folding adj into W0, a constant
per-weight scale): M is resharded to Mp[j, t_local, b] fp16 so the
contraction dim j lands on SBUF partitions and b is the contiguous matmul
free dim; weights are pre-transposed fp16 lhsT layouts, pair/quad-packed
across partitions.

Device pipeline per 8-t block: DMA Mp slabs (split across both HWDGE
rings) -> DVE in-place multiply by broadcast x^T[j,b] -> per t-PAIR:
L0/L1 matmuls col/row-tiled so two 64-wide nodes use both halves of the
PE array concurrently, ScalarE Lrelu(+bias) handles both nodes in one op;
L2 packs 4 nodes (M=2 each) per PSUM tile; DVE adds b2; GPSIMD DMAs out.
"""

import numpy as np

D, HID, P, BS = 256, 64, 2, 1024
NCORES = 8
TLOC = D // NCORES          # 32 t's per core
JC = 2                      # j split into 2 chunks of 128 partitions
JP = 128
TBLK = 4                    # t's per M slab
NPAIR = TLOC // 2
NQUAD = TLOC // 4

TRACE = False
TRACE_CORES = None
LAST_RESULTS = None


# ---------------------------------------------------------------------------
# Toolchain workarounds: this container's walrus accepts at most ONE sync
# wait per instruction; Tile emits several (worst on the tail drain).
# ---------------------------------------------------------------------------
def _install_patches():
    import bass_rust
    import concourse.tile as tile
    from concourse.vector_clock import ScopedClock

    if getattr(tile.TileContext, "_drain_patch_installed", False):
        return

    def _patched_drain_and_barrier(self, tick_clock, wait_clock):
        probe = self.nc.sync.nop()
        wait_clock.add_sem_waits(
            probe.ins, ScopedClock({None: tick_clock.global_clock})
        )
        si = probe.ins.sync_info
        waits = list(si.on_wait) if si is not None else []
        if len(waits) > 1:
            probe.ins.sync_info = bass_rust.SyncInfo(
                on_wait=[], on_update=list(si.on_update)
            )
            handles = {h.name: h for h in self.sems.allocated().values()}
            # spread the waits over all engines so they resolve in
            # parallel; the all_engine_barrier below joins them
            engs = [self.nc.sync, self.nc.vector, self.nc.scalar,
                    self.nc.gpsimd, self.nc.tensor]
            for i, w in enumerate(waits):
                engs[i % len(engs)].wait_ge(handles[w.ant_name], w.wait_value)
        drain_inst = self.nc.sync.drain()
        wait_clock.add_sem_waits(
            drain_inst.ins, ScopedClock({None: tick_clock.global_clock})
        )
        dsi = drain_inst.ins.sync_info
        if dsi is not None and len(dsi.on_wait) > 1:
            drain_inst.ins.sync_info = bass_rust.SyncInfo(
                on_wait=[], on_update=list(dsi.on_update)
            )
        self.nc.all_engine_barrier()
        assert self.sems is not None
        popped = self.nc._tile_sem_poison_stack.pop()
        assert popped is self._sem_poison
        self.nc.clear_and_free_semaphores(list(self.sems.allocated().values()))
        self.nc.all_engine_barrier()

    tile.TileContext._drain_and_barrier = _patched_drain_and_barrier
    tile.TileContext._drain_patch_installed = True


def _split_multiwait_instructions(nc):
    """Move extra sync waits onto single-wait NoOps inserted just before,
    on the same engine — ordering semantics preserved."""
    import bass_rust

    k = 0
    for fn in nc.m.functions:
        for bb in fn.blocks:
            insts = bb.instructions
            out = []
            changed = False
            for inst in insts:
                si = inst.sync_info
                waits = list(si.on_wait) if si is not None else []
                if len(waits) > 1:
                    changed = True
                    for w in waits[:-1]:
                        nop = bass_rust.InstNoOp(
                            name=f"mwsplit_{k}", ins=[], outs=[]
                        )
                        k += 1
                        nop.engine = inst.engine
                        nop.sync_info = bass_rust.SyncInfo(
                            on_wait=[w], on_update=[]
                        )
                        out.append(nop)
                    inst.sync_info = bass_rust.SyncInfo(
                        on_wait=[waits[-1]], on_update=list(si.on_update)
                    )
                out.append(inst)
            if changed:
                bb.instructions = out


def _install_ntff_hook():
    import sys
    import types

    try:
        from antenv.axon_hooks import get_axon_ntff_profile_hook  # noqa: F401

        return True
    except ImportError:
        pass
    mod = types.ModuleType("antenv.axon_hooks")
    _hook = [None]
    mod.set_axon_ntff_profile_hook = lambda h: _hook.__setitem__(0, h)
    mod.get_axon_ntff_profile_hook = lambda: _hook[0]
    sys.modules["antenv.axon_hooks"] = mod
    import antenv

    antenv.axon_hooks = mod
    try:
        from trn_agent_boot.trn_boot import _ntff_profile_via_ctypes

        mod.set_axon_ntff_profile_hook(
            _ntff_profile_via_ctypes("/opt/axon/libaxon_pjrt.so")
        )
        return True
    except Exception:
        return False


# ---------------------------------------------------------------------------
# Device program
# ---------------------------------------------------------------------------
_PROGRAM = None


def _build_program():
    import concourse.bass as bass
    import concourse.mybir as mybir
    import concourse.tile as tile
    from concourse.alu_op_type import AluOpType

    _install_patches()

    f32 = mybir.dt.float32
    f16 = mybir.dt.float16

    nc = bass.Bass()
    mp = nc.dram_tensor("mp", [JC, TLOC // TBLK, JP, TBLK, BS], f16, kind="ExternalInput")
    w0 = nc.dram_tensor("w0", [JP, JC, TLOC, HID], f16, kind="ExternalInput")
    w1 = nc.dram_tensor("w1", [JP, NPAIR, HID], f16, kind="ExternalInput")
    w2 = nc.dram_tensor("w2", [JP, NPAIR, P], f16, kind="ExternalInput")
    xt = nc.dram_tensor("xt", [JC, JP, BS], f16, kind="ExternalInput")
    b0 = nc.dram_tensor("b0", [JP, NPAIR], f32, kind="ExternalInput")
    b1 = nc.dram_tensor("b1", [JP, NPAIR], f32, kind="ExternalInput")
    b2 = nc.dram_tensor("b2", [JP, NQUAD], f32, kind="ExternalInput")
    out = nc.dram_tensor("out", [TLOC, P, BS], f32, kind="ExternalOutput")

    NBT = TLOC // TBLK  # number of t blocks
    Lrelu = mybir.ActivationFunctionType.Lrelu
    NS = [slice(0, 512), slice(512, 1024)]

    with tile.TileContext(nc) as tc:
        with (
            tc.tile_pool(name="consts", bufs=1) as consts,
            tc.tile_pool(name="mslab", bufs=12) as mpool,
            tc.tile_pool(name="htiles", bufs=4) as hpool,
            tc.tile_pool(name="otiles", bufs=3) as opool,
            tc.tile_pool(name="ps0", bufs=2, space="PSUM") as ps0pool,
            tc.tile_pool(name="ps1", bufs=1, space="PSUM") as ps1pool,
            tc.tile_pool(name="ps2", bufs=1, space="PSUM") as ps2pool,
        ):
            # constants first, split across both HWDGE rings so compute
            # can start as soon as possible; M slabs queue up behind them
            xt_sb = []
            for jc in range(JC):
                t_ = consts.tile([JP, BS], f16, name=f"xt{jc}")
                eng = nc.sync if jc == 0 else nc.scalar
                eng.dma_start(out=t_[:], in_=xt[jc, :, :])
                xt_sb.append(t_)
            w0_sb = consts.tile([JP, JC, TLOC, HID], f16)
            nc.sync.dma_start(out=w0_sb[:, 0], in_=w0[:, 0])
            nc.scalar.dma_start(out=w0_sb[:, 1], in_=w0[:, 1])
            w1_sb = consts.tile([JP, NPAIR, HID], f16)
            nc.sync.dma_start(out=w1_sb[:], in_=w1[:, :, :])
            w2_sb = consts.tile([JP, NPAIR, P], f16)
            nc.scalar.dma_start(out=w2_sb[:], in_=w2[:, :, :])
            b0_sb = consts.tile([JP, NPAIR], f32)
            nc.sync.dma_start(out=b0_sb[:], in_=b0[:, :])
            b1_sb = consts.tile([JP, NPAIR], f32)
            nc.scalar.dma_start(out=b1_sb[:], in_=b1[:, :])
            b2_sb = consts.tile([JP, NQUAD], f32)
            nc.sync.dma_start(out=b2_sb[:], in_=b2[:, :])

            for tb in range(NBT):
                t0 = tb * TBLK
                mts = []
                for jc in range(JC):
                    mt = mpool.tile([JP, TBLK, BS], f16, tag="mslab")
                    # split the big M streams across both HWDGE rings
                    eng = nc.sync if jc == 0 else nc.scalar
                    eng.dma_start(out=mt[:], in_=mp[jc, tb])
                    mts.append(mt)
                # fold x in (in place): mt[j, t, b] *= x^T[j, b]
                for jc in range(JC):
                    nc.vector.tensor_tensor(
                        mts[jc][:],
                        mts[jc][:],
                        xt_sb[jc][:].unsqueeze(1).broadcast_to((JP, TBLK, BS)),
                        op=AluOpType.mult,
                    )
                h2s = {}
                for pr in range(TBLK // 2):
                    p = tb * (TBLK // 2) + pr       # global pair index
                    te = t0 + 2 * pr                # even t (local)
                    to = te + 1                     # odd t (local)
                    re, ro = 2 * pr, 2 * pr + 1     # row indices in mts
                    ps0 = ps0pool.tile([JP, BS], f32, tag="ps0")
                    for ns in NS:
                        for jc in range(JC):
                            nc.tensor.matmul(
                                ps0[0:HID, ns],
                                w0_sb[:, jc, te, :],
                                mts[jc][:, re, ns],
                                start=(jc == 0),
                                stop=(jc == JC - 1),
                            )
                        for jc in range(JC):
                            nc.tensor.matmul(
                                ps0[HID:JP, ns],
                                w0_sb[:, jc, to, :],
                                mts[jc][:, ro, ns],
                                start=(jc == 0),
                                stop=(jc == JC - 1),
                            )
                    h1 = hpool.tile([JP, BS], f16, tag="h1")
                    nc.scalar.activation(
                        h1[:], ps0[:], Lrelu,
                        bias=b0_sb[:, p : p + 1], scale=1.0, alpha=0.01,
                    )
                    ps1 = ps1pool.tile([JP, BS], f32, tag="ps1")
                    for ns in NS:
                        nc.tensor.matmul(
                            ps1[0:HID, ns], w1_sb[0:HID, p, :], h1[0:HID, ns],
                            start=True, stop=True,
                        )
                        nc.tensor.matmul(
                            ps1[HID:JP, ns], w1_sb[HID:JP, p, :], h1[HID:JP, ns],
                            start=True, stop=True,
                        )
                    h2 = hpool.tile([JP, BS], f16, tag="h2")
                    nc.scalar.activation(
                        h2[:], ps1[:], Lrelu,
                        bias=b1_sb[:, p : p + 1], scale=1.0, alpha=0.01,
                    )
                    h2s[pr] = h2
                for q in range(TBLK // 4):
                    qg = tb * (TBLK // 4) + q       # global quad index
                    ps2 = ps2pool.tile([JP, BS], f32, tag="ps2")
                    for c in range(4):
                        pr = 2 * q + c // 2
                        pglob = tb * (TBLK // 2) + pr
                        base = HID * (c % 2)
                        col = 32 * c
                        for ns in NS:
                            nc.tensor.matmul(
                                ps2[col : col + P, ns],
                                w2_sb[base : base + HID, pglob, :],
                                h2s[pr][base : base + HID, ns],
                                start=True, stop=True,
                                tile_position=(base, col),
                            )
                    osb = opool.tile([JP, BS], f32, tag="osb")
                    nc.vector.tensor_scalar_add(
                        osb[:], ps2[:], b2_sb[:, qg : qg + 1]
                    )
                    for c in range(4):
                        t = 4 * qg + c
                        nc.gpsimd.dma_start(
                            out=out[t, :, :], in_=osb[32 * c : 32 * c + P, :]
                        )
    _split_multiwait_instructions(nc)
    return nc


def _get_program():
    global _PROGRAM
    if _PROGRAM is None:
        _PROGRAM = _build_program()
    return _PROGRAM


# ---------------------------------------------------------------------------
# Host wrapper
# ---------------------------------------------------------------------------
def kernel(x, M, adj, W0, b0, W1, b1, W2, b2):
    global LAST_RESULTS
    from concourse import bass_utils

    x = np.asarray(x, np.float32)
    M = np.asarray(M, np.float32)
    adj = np.asarray(adj, np.float32)
    W0 = np.asarray(W0, np.float32)
    b0 = np.asarray(b0, np.float32)
    W1 = np.asarray(W1, np.float32)
    b1 = np.asarray(b1, np.float32)
    W2 = np.asarray(W2, np.float32)
    b2 = np.asarray(b2, np.float32)

    xt_full = np.ascontiguousarray(x.T.astype(np.float16)).reshape(JC, JP, BS)

    def pack_pairs(a):
        # a: (TLOC, HID, ...) per-t lhsT rows (j=HID) -> (128, NPAIR, ...)
        # rows 0:64 = even t, rows 64:128 = odd t
        ev, od = a[0::2], a[1::2]           # (NPAIR, HID, ...)
        return np.concatenate([ev, od], axis=1).transpose(
            (1, 0) + tuple(range(2, a.ndim))
        )

    in_maps = []
    for c in range(NCORES):
        tsl = slice(c * TLOC, (c + 1) * TLOC)
        mp = np.ascontiguousarray(
            M[:, :, tsl]
            .transpose(1, 2, 0)
            .reshape(JC, JP, TLOC // TBLK, TBLK, BS)
            .transpose(0, 2, 1, 3, 4)
        ).astype(np.float16)
        # fold adj into W0: W0eff[t,i,j] = W0[t,i,j] * adj[j,t]
        w0eff = W0[tsl] * adj.T[tsl][:, None, :]  # (TLOC, HID, D)
        w0l = np.ascontiguousarray(
            w0eff.transpose(2, 0, 1).reshape(JC, JP, TLOC, HID).transpose(1, 0, 2, 3)
        ).astype(np.float16)
        w1t = W1[tsl].transpose(0, 2, 1)          # (TLOC, j, i)
        w2t = W2[tsl].transpose(0, 2, 1)          # (TLOC, j, p)
        w1l = np.ascontiguousarray(pack_pairs(w1t)).astype(np.float16)
        w2l = np.ascontiguousarray(pack_pairs(w2t)).astype(np.float16)
        b0t = b0[tsl]                             # (TLOC, HID)
        b1t = b1[tsl]
        b0l = np.ascontiguousarray(pack_pairs(b0t[:, :, None])[:, :, 0])
        b1l = np.ascontiguousarray(pack_pairs(b1t[:, :, None])[:, :, 0])
        b2t = b2[tsl]                             # (TLOC, P)
        b2l = np.zeros((JP, NQUAD), np.float32)
        for t in range(TLOC):
            qg, cc = divmod(t, 4)
            b2l[32 * cc : 32 * cc + P, qg] = b2t[t]
        in_maps.append(
            {
                "mp": mp,
                "w0": w0l,
                "w1": w1l,
                "w2": w2l,
                "xt": xt_full,
                "b0": b0l,
                "b1": b1l,
                "b2": b2l,
            }
        )

    nc = _get_program()
    kw = {}
    if TRACE:
        _install_ntff_hook()
        kw["trace"] = True
        if TRACE_CORES is not None:
            kw["trace_cores"] = TRACE_CORES
    res = bass_utils.run_bass_kernel_spmd(
        nc, in_maps, core_ids=list(range(NCORES)), **kw
    )
    LAST_RESULTS = res

    out = np.empty((BS, D, P), np.float32)
    for c in range(NCORES):
        tsl = slice(c * TLOC, (c + 1) * TLOC)
        out[:, tsl, :] = res.results[c]["out"].transpose(2, 0, 1)
    return out


# revision 17
# speedup vs baseline: 1.0683x; 1.0683x over previous
"""Trainium2 Bass kernel for nn_BaseMLP (per-node GNN message-passing MLP).

Reference computation (D=256 nodes, HID=64, P=2, BS=1024):
    xmask[b,j,t] = M[b,j,t] * adj[j,t] * x[b,j]
    h   = lrelu(einsum('tij,bjt->bti', W0, xmask) + b0)
    h   = lrelu(einsum('tij,btj->bti', W1, h) + b1)
    out =        einsum('tij,btj->bti', W2, h) + b2

Sharding: model-parallel over the target-node dim t — each of the 8 cores
owns 32 t's. Per-core M traffic matches batch sharding (M/8) while
replicated-weight traffic drops 8x.

Host-side prep (layout + dtype only, plus folding adj into W0, a constant
per-weight scale): M is resharded to Mp[j, t_local, b] fp16 so the
contraction dim j lands on SBUF partitions and b is the contiguous matmul
free dim; weights are pre-transposed fp16 lhsT layouts, pair/quad-packed
across partitions.

Device pipeline per 8-t block: DMA Mp slabs (split across both HWDGE
rings) -> DVE in-place multiply by broadcast x^T[j,b] -> per t-PAIR:
L0/L1 matmuls col/row-tiled so two 64-wide nodes use both halves of the
PE array concurrently, ScalarE Lrelu(+bias) handles both nodes in one op;
L2 packs 4 nodes (M=2 each) per PSUM tile; DVE adds b2; GPSIMD DMAs out.
"""

import numpy as np

D, HID, P, BS = 256, 64, 2, 1024
NCORES = 8
TLOC = D // NCORES          # 32 t's per core
JC = 2                      # j split into 2 chunks of 128 partitions
JP = 128
TBLK = 4                    # t's per M slab
NPAIR = TLOC // 2
NQUAD = TLOC // 4

TRACE = False
TRACE_CORES = None
LAST_RESULTS = None


# ---------------------------------------------------------------------------
# Toolchain workarounds: this container's walrus accepts at most ONE sync
# wait per instruction; Tile emits several (worst on the tail drain).
# ---------------------------------------------------------------------------
def _install_patches():
    import bass_rust
    import concourse.tile as tile
    from concourse.vector_clock import ScopedClock

    if getattr(tile.TileContext, "_drain_patch_installed", False):
        return

    def _patched_drain_and_barrier(self, tick_clock, wait_clock):
        probe = self.nc.sync.nop()
        wait_clock.add_sem_waits(
            probe.ins, ScopedClock({None: tick_clock.global_clock})
        )
        si = probe.ins.sync_info
        waits = list(si.on_wait) if si is not None else []
        if len(waits) > 1:
            probe.ins.sync_info = bass_rust.SyncInfo(
                on_wait=[], on_update=list(si.on_update)
            )
            handles = {h.name: h for h in self.sems.allocated().values()}
            # spread the waits over all engines so they resolve in
            # parallel; the all_engine_barrier below joins them
            engs = [self.nc.sync, self.nc.vector, self.nc.scalar,
                    self.nc.gpsimd, self.nc.tensor]
            for i, w in enumerate(waits):
                engs[i % len(engs)].wait_ge(handles[w.ant_name], w.wait_value)
        drain_inst = self.nc.sync.drain()
        wait_clock.add_sem_waits(
            drain_inst.ins, ScopedClock({None: tick_clock.global_clock})
        )
        dsi = drain_inst.ins.sync_info
        if dsi is not None and len(dsi.on_wait) > 1:
            drain_inst.ins.sync_info = bass_rust.SyncInfo(
                on_wait=[], on_update=list(dsi.on_update)
            )
        self.nc.all_engine_barrier()
        assert self.sems is not None
        popped = self.nc._tile_sem_poison_stack.pop()
        assert popped is self._sem_poison
        self.nc.clear_and_free_semaphores(list(self.sems.allocated().values()))
        self.nc.all_engine_barrier()

    tile.TileContext._drain_and_barrier = _patched_drain_and_barrier
    tile.TileContext._drain_patch_installed = True


def _split_multiwait_instructions(nc):
    """Move extra sync waits onto single-wait NoOps inserted just before,
    on the same engine — ordering semantics preserved."""
    import bass_rust

    k = 0
    for fn in nc.m.functions:
        for bb in fn.blocks:
            insts = bb.instructions
            out = []
            changed = False
            for inst in insts:
                si = inst.sync_info
                waits = list(si.on_wait) if si is not None else []
                if len(waits) > 1:
                    changed = True
                    for w in waits[:-1]:
                        nop = bass_rust.InstNoOp(
                            name=f"mwsplit_{k}", ins=[], outs=[]
                        )
                        k += 1
                        nop.engine = inst.engine
                        nop.sync_info = bass_rust.SyncInfo(
                            on_wait=[w], on_update=[]
                        )
                        out.append(nop)
                    inst.sync_info = bass_rust.SyncInfo(
                        on_wait=[waits[-1]], on_update=list(si.on_update)
                    )
                out.append(inst)
            if changed:
                bb.instructions = out


def _install_ntff_hook():
    import sys
    import types

    try:
        from antenv.axon_hooks import get_axon_ntff_profile_hook  # noqa: F401

        return True
    except ImportError:
        pass
    mod = types.ModuleType("antenv.axon_hooks")
    _hook = [None]
    mod.set_axon_ntff_profile_hook = lambda h: _hook.__setitem__(0, h)
    mod.get_axon_ntff_profile_hook = lambda: _hook[0]
    sys.modules["antenv.axon_hooks"] = mod
    import antenv

    antenv.axon_hooks = mod
    try:
        from trn_agent_boot.trn_boot import _ntff_profile_via_ctypes

        mod.set_axon_ntff_profile_hook(
            _ntff_profile_via_ctypes("/opt/axon/libaxon_pjrt.so")
        )
        return True
    except Exception:
        return False


# ---------------------------------------------------------------------------
# Device program
# ---------------------------------------------------------------------------
_PROGRAM = None


def _build_program():
    import concourse.bass as bass
    import concourse.mybir as mybir
    import concourse.tile as tile
    from concourse.alu_op_type import AluOpType

    _install_patches()

    f32 = mybir.dt.float32
    f16 = mybir.dt.float16

    nc = bass.Bass()
    mp = nc.dram_tensor("mp", [JC, TLOC // TBLK, JP, TBLK, BS], f16, kind="ExternalInput")
    w0 = nc.dram_tensor("w0", [JP, JC, TLOC, HID], f16, kind="ExternalInput")
    w1 = nc.dram_tensor("w1", [JP, NPAIR, HID], f16, kind="ExternalInput")
    w2 = nc.dram_tensor("w2", [JP, NPAIR, P], f16, kind="ExternalInput")
    xt = nc.dram_tensor("xt", [JC, JP, BS], f16, kind="ExternalInput")
    b0 = nc.dram_tensor("b0", [JP, NPAIR], f32, kind="ExternalInput")
    b1 = nc.dram_tensor("b1", [JP, NPAIR], f32, kind="ExternalInput")
    b2 = nc.dram_tensor("b2", [JP, NQUAD], f32, kind="ExternalInput")
    out = nc.dram_tensor("out", [TLOC, P, BS], f32, kind="ExternalOutput")

    NBT = TLOC // TBLK  # number of t blocks
    Lrelu = mybir.ActivationFunctionType.Lrelu
    NS = [slice(0, 512), slice(512, 1024)]

    with tile.TileContext(nc) as tc:
        with (
            tc.tile_pool(name="consts", bufs=1) as consts,
            tc.tile_pool(name="mslab", bufs=12) as mpool,
            tc.tile_pool(name="htiles", bufs=4) as hpool,
            tc.tile_pool(name="otiles", bufs=3) as opool,
            tc.tile_pool(name="ps0", bufs=2, space="PSUM") as ps0pool,
            tc.tile_pool(name="ps12", bufs=2, space="PSUM") as ps12pool,
        ):
            # xt first (needed by the very first DVE op), then the first
            # M slab on each ring, then the bulk weights, then the rest of
            # the M stream — so compute starts as early as possible
            xt_sb = []
            for jc in range(JC):
                t_ = consts.tile([JP, BS], f16, name=f"xt{jc}")
                eng = nc.sync if jc == 0 else nc.scalar
                eng.dma_start(out=t_[:], in_=xt[jc, :, :])
                xt_sb.append(t_)
            mts0 = []
            for jc in range(JC):
                mt = mpool.tile([JP, TBLK, BS], f16, tag="mslab")
                eng = nc.sync if jc == 0 else nc.scalar
                eng.dma_start(out=mt[:], in_=mp[jc, 0])
                mts0.append(mt)
            w0_sb = consts.tile([JP, JC, TLOC, HID], f16)
            nc.sync.dma_start(out=w0_sb[:, 0], in_=w0[:, 0])
            nc.scalar.dma_start(out=w0_sb[:, 1], in_=w0[:, 1])
            w1_sb = consts.tile([JP, NPAIR, HID], f16)
            nc.sync.dma_start(out=w1_sb[:], in_=w1[:, :, :])
            w2_sb = consts.tile([JP, NPAIR, P], f16)
            nc.scalar.dma_start(out=w2_sb[:], in_=w2[:, :, :])
            b0_sb = consts.tile([JP, NPAIR], f32)
            nc.sync.dma_start(out=b0_sb[:], in_=b0[:, :])
            b1_sb = consts.tile([JP, NPAIR], f32)
            nc.scalar.dma_start(out=b1_sb[:], in_=b1[:, :])
            b2_sb = consts.tile([JP, NQUAD], f32)
            nc.sync.dma_start(out=b2_sb[:], in_=b2[:, :])

            for tb in range(NBT):
                t0 = tb * TBLK
                if tb == 0:
                    mts = mts0
                else:
                    mts = []
                    for jc in range(JC):
                        mt = mpool.tile([JP, TBLK, BS], f16, tag="mslab")
                        # split the big M streams across both HWDGE rings
                        eng = nc.sync if jc == 0 else nc.scalar
                        eng.dma_start(out=mt[:], in_=mp[jc, tb])
                        mts.append(mt)
                # fold x in (in place): mt[j, t, b] *= x^T[j, b]
                for jc in range(JC):
                    nc.vector.tensor_tensor(
                        mts[jc][:],
                        mts[jc][:],
                        xt_sb[jc][:].unsqueeze(1).broadcast_to((JP, TBLK, BS)),
                        op=AluOpType.mult,
                    )
                h2s = {}
                for pr in range(TBLK // 2):
                    p = tb * (TBLK // 2) + pr       # global pair index
                    te = t0 + 2 * pr                # even t (local)
                    to = te + 1                     # odd t (local)
                    re, ro = 2 * pr, 2 * pr + 1     # row indices in mts
                    ps0 = ps0pool.tile([JP, BS], f32, tag="ps0")
                    for ns in NS:
                        for jc in range(JC):
                            nc.tensor.matmul(
                                ps0[0:HID, ns],
                                w0_sb[:, jc, te, :],
                                mts[jc][:, re, ns],
                                start=(jc == 0),
                                stop=(jc == JC - 1),
                            )
                        for jc in range(JC):
                            nc.tensor.matmul(
                                ps0[HID:JP, ns],
                                w0_sb[:, jc, to, :],
                                mts[jc][:, ro, ns],
                                start=(jc == 0),
                                stop=(jc == JC - 1),
                            )
                    h1 = hpool.tile([JP, BS], f16, tag="h1")
                    nc.scalar.activation(
                        h1[:], ps0[:], Lrelu,
                        bias=b0_sb[:, p : p + 1], scale=1.0, alpha=0.01,
                    )
                    ps1 = ps12pool.tile([JP, BS], f32, tag="ps12")
                    for ns in NS:
                        nc.tensor.matmul(
                            ps1[0:HID, ns], w1_sb[0:HID, p, :], h1[0:HID, ns],
                            start=True, stop=True,
                        )
                        nc.tensor.matmul(
                            ps1[HID:JP, ns], w1_sb[HID:JP, p, :], h1[HID:JP, ns],
                            start=True, stop=True,
                        )
                    h2 = hpool.tile([JP, BS], f16, tag="h2")
                    nc.scalar.activation(
                        h2[:], ps1[:], Lrelu,
                        bias=b1_sb[:, p : p + 1], scale=1.0, alpha=0.01,
                    )
                    h2s[pr] = h2
                for q in range(TBLK // 4):
                    qg = tb * (TBLK // 4) + q       # global quad index
                    ps2 = ps12pool.tile([JP, BS], f32, tag="ps12")
                    for c in range(4):
                        pr = 2 * q + c // 2
                        pglob = tb * (TBLK // 2) + pr
                        base = HID * (c % 2)
                        col = 32 * c
                        for ns in NS:
                            nc.tensor.matmul(
                                ps2[col : col + P, ns],
                                w2_sb[base : base + HID, pglob, :],
                                h2s[pr][base : base + HID, ns],
                                start=True, stop=True,
                                tile_position=(base, col),
                            )
                    osb = opool.tile([JP, BS], f32, tag="osb")
                    nc.vector.tensor_scalar_add(
                        osb[:], ps2[:], b2_sb[:, qg : qg + 1]
                    )
                    for c in range(4):
                        t = 4 * qg + c
                        nc.gpsimd.dma_start(
                            out=out[t, :, :], in_=osb[32 * c : 32 * c + P, :]
                        )
    _split_multiwait_instructions(nc)
    return nc


def _get_program():
    global _PROGRAM
    if _PROGRAM is None:
        _PROGRAM = _build_program()
    return _PROGRAM


# ---------------------------------------------------------------------------
# Host wrapper
# ---------------------------------------------------------------------------
def kernel(x, M, adj, W0, b0, W1, b1, W2, b2):
    global LAST_RESULTS
    from concourse import bass_utils

    x = np.asarray(x, np.float32)
    M = np.asarray(M, np.float32)
    adj = np.asarray(adj, np.float32)
    W0 = np.asarray(W0, np.float32)
    b0 = np.asarray(b0, np.float32)
    W1 = np.asarray(W1, np.float32)
    b1 = np.asarray(b1, np.float32)
    W2 = np.asarray(W2, np.float32)
    b2 = np.asarray(b2, np.float32)

    xt_full = np.ascontiguousarray(x.T.astype(np.float16)).reshape(JC, JP, BS)

    def pack_pairs(a):
        # a: (TLOC, HID, ...) per-t lhsT rows (j=HID) -> (128, NPAIR, ...)
        # rows 0:64 = even t, rows 64:128 = odd t
        ev, od = a[0::2], a[1::2]           # (NPAIR, HID, ...)
        return np.concatenate([ev, od], axis=1).transpose(
            (1, 0) + tuple(range(2, a.ndim))
        )

    in_maps = []
    for c in range(NCORES):
        tsl = slice(c * TLOC, (c + 1) * TLOC)
        mp = np.ascontiguousarray(
            M[:, :, tsl]
            .transpose(1, 2, 0)
            .reshape(JC, JP, TLOC // TBLK, TBLK, BS)
            .transpose(0, 2, 1, 3, 4)
        ).astype(np.float16)
        # fold adj into W0: W0eff[t,i,j] = W0[t,i,j] * adj[j,t]
        w0eff = W0[tsl] * adj.T[tsl][:, None, :]  # (TLOC, HID, D)
        w0l = np.ascontiguousarray(
            w0eff.transpose(2, 0, 1).reshape(JC, JP, TLOC, HID).transpose(1, 0, 2, 3)
        ).astype(np.float16)
        w1t = W1[tsl].transpose(0, 2, 1)          # (TLOC, j, i)
        w2t = W2[tsl].transpose(0, 2, 1)          # (TLOC, j, p)
        w1l = np.ascontiguousarray(pack_pairs(w1t)).astype(np.float16)
        w2l = np.ascontiguousarray(pack_pairs(w2t)).astype(np.float16)
        b0t = b0[tsl]                             # (TLOC, HID)
        b1t = b1[tsl]
        b0l = np.ascontiguousarray(pack_pairs(b0t[:, :, None])[:, :, 0])
        b1l = np.ascontiguousarray(pack_pairs(b1t[:, :, None])[:, :, 0])
        b2t = b2[tsl]                             # (TLOC, P)
        b2l = np.zeros((JP, NQUAD), np.float32)
        for t in range(TLOC):
            qg, cc = divmod(t, 4)
            b2l[32 * cc : 32 * cc + P, qg] = b2t[t]
        in_maps.append(
            {
                "mp": mp,
                "w0": w0l,
                "w1": w1l,
                "w2": w2l,
                "xt": xt_full,
                "b0": b0l,
                "b1": b1l,
                "b2": b2l,
            }
        )

    nc = _get_program()
    kw = {}
    if TRACE:
        _install_ntff_hook()
        kw["trace"] = True
        if TRACE_CORES is not None:
            kw["trace_cores"] = TRACE_CORES
    res = bass_utils.run_bass_kernel_spmd(
        nc, in_maps, core_ids=list(range(NCORES)), **kw
    )
    LAST_RESULTS = res

    out = np.empty((BS, D, P), np.float32)
    for c in range(NCORES):
        tsl = slice(c * TLOC, (c + 1) * TLOC)
        out[:, tsl, :] = res.results[c]["out"].transpose(2, 0, 1)
    return out
